# revision 47
# baseline (speedup 1.0000x reference)
"""Trainium2 Bass kernel: causal self-attention (B=2, T=2048, C=1024, H=16, Dh=64).

Sharding: 8 cores = 2 (batch) x 4 (head groups of 4 heads).  Each core gets
x[b] plus the W_qkv rows / W_proj columns for its heads, computes the full
attention + a partial output projection for its batch, and the host sums the
4 partials per batch (tensor-parallel unshard).

All matmuls run in bf16 with f32 PSUM accumulation.  x is passed transposed
(xT = x[b].T) so that:
  qT, kT = Wq @ xT, Wk @ xT     (head dim on partitions)  -- no transposes
  v      = xT.T @ WvT           (natural [T, d] layout)
  S^T    = kT_h(tile).T @ qT_h  ([k, q] layout, 128x512 blocks, the two
           heads of a pair row-tiled into array halves -> concurrent)
  exp on ScalarE (logits are bounded, no max pass needed); causal masking by
  computing only the live columns of each block plus one multiplicative
  [128,128] triangle mask on the diagonal subtile (gpsimd); row sums via a
  ones column appended to V (P@[V|1] accumulates y^T and the softmax
  denominators in one PSUM tile).
  out_partial = y^T.T @ WpT   (f32, DMA'd out).

Schedule: the PE executes its queue IN ORDER, so QKV tiles are emitted
through a deadline-ordered work queue sprinkled one-per-iteration into the
attention i-loops (plus previous-chunk projection tiles), never as a bulk
phase.  This keeps the PE continuously busy from ~10us on: the S->exp->Y
handoff bubbles are filled with independent QKV/projection matmuls, which
also keeps the HAM activity monitor at K=8/8 (full 2.4GHz clock) instead of
oscillating into the 1.2GHz throttle state.  Y matmuls are emitted one
iteration behind their exp (software pipelining) so the in-order PE queue
never parks on an unfinished ACTIVATE.

Softmax normalization: denominator rows go through a DRAM round-trip that
re-spreads the 512 q-columns over 8 SBUF partitions, so the iterative-
divide DVE RECIPROCAL runs 8 lanes wide (0.9us for two heads vs 3.3us per
head single-lane); the reciprocal row is broadcast back over 128 partitions
by a stride-0 DRAM-read DMA and applied with one DVE multiply.  The last
chunk instead uses a zero-DMA chain (single-lane reciprocal + PE outer-
product broadcast) because the final projection burst gates the kernel end,
and throwaway warm-filler matmuls bridge that chain so the burst runs at
full clock.

ScalarE runs nothing but Exp (plus two DMA descriptor issues per projection
pair); every PSUM eviction lives on the DVE.  Diagonal-block exps cover
both disjoint live spans with one 3D-AP ACTIVATE, halving the +352-cycle
pipeline-fill cost on the diagonal.
"""
import sys
import types

import numpy as np
import ml_dtypes

_BF16 = ml_dtypes.bfloat16


def _install_ntff_hook():
    """Provide antenv.axon_hooks so run_bass_kernel_spmd(trace=True) works."""
    if "antenv.axon_hooks" in sys.modules:
        return
    mod = types.ModuleType("antenv.axon_hooks")
    mod._hook = None

    def set_axon_ntff_profile_hook(h):
        mod._hook = h

    def get_axon_ntff_profile_hook():
        return mod._hook

    mod.set_axon_ntff_profile_hook = set_axon_ntff_profile_hook
    mod.get_axon_ntff_profile_hook = get_axon_ntff_profile_hook
    sys.modules["antenv.axon_hooks"] = mod
    try:
        import antenv

        antenv.axon_hooks = mod
    except Exception:
        pass
    try:
        from trn_agent_boot.trn_boot import _ntff_profile_via_ctypes

        mod.set_axon_ntff_profile_hook(
            _ntff_profile_via_ctypes("/opt/axon/libaxon_pjrt.so")
        )
    except Exception:
        pass


_install_ntff_hook()

import concourse.bacc as bacc
import concourse.mybir as mybir
from concourse import bass_utils
from concourse.tile import TileContext

# no network bucket in this container; keep artifacts local
bass_utils.upload_artifacts = lambda tmpdir: tmpdir

BF16 = mybir.dt.bfloat16
F32 = mybir.dt.float32

B, T, C = 2, 2048, 1024
H, D = 16, 64
HL = 4            # heads per core
OL = HL * D       # 256 local qkv output dim
P = 128
KC = C // P       # 8 contraction chunks
NQT = T // P      # 16 q/k 128-tiles
NQC = T // 512    # 4 q 512-chunks
VA = D + 1        # v columns per head incl. ones column (65)

_nc_cache = None


def _build_nc():
    nc = bacc.Bacc("TRN2", target_bir_lowering=False, debug=False, num_devices=8)

    # all inputs arrive pre-arranged in SBUF-image layout [128, X] so every
    # input DMA moves multi-KB contiguous runs per partition row.
    xT = nc.declare_dram_parameter("xT", [P, KC * T], BF16, isOutput=False)
    wqT = nc.declare_dram_parameter("wqT", [P, KC * OL], BF16, isOutput=False)
    wkT = nc.declare_dram_parameter("wkT", [P, KC * OL], BF16, isOutput=False)
    wvT = nc.declare_dram_parameter("wvT", [P, KC * OL], BF16, isOutput=False)
    wpT = nc.declare_dram_parameter("wpT", [P, 2 * C], BF16, isOutput=False)
    mk = nc.declare_dram_parameter("mask_tri", [P, P], BF16, isOutput=False)
    out = nc.declare_dram_parameter("out", [T, C], F32, isOutput=True)

    Exp = mybir.ActivationFunctionType.Exp

    with TileContext(nc) as tc:
        with tc.tile_pool(name="const", bufs=1) as const, \
             tc.tile_pool(name="misc", bufs=3) as misc, \
             tc.tile_pool(name="att", bufs=8) as att, \
             tc.tile_pool(name="outp", bufs=6) as outp:
            xT_sb = const.tile([P, KC * T], BF16, name="xT_sb")
            wq_sb = const.tile([P, KC * OL], BF16, name="wq_sb")
            wk_sb = const.tile([P, KC * OL], BF16, name="wk_sb")
            wv_sb = const.tile([P, KC * OL], BF16, name="wv_sb")
            wp_sb = const.tile([P, 2 * C], BF16, name="wp_sb")
            mk_sb = const.tile([P, P], BF16, name="mk_sb")
            ones_sb = const.tile([1, P], BF16, name="ones_sb")
            qT_sb = const.tile([P, 2 * T], BF16, name="qT_sb")
            kT_sb = const.tile([P, 2 * T], BF16, name="kT_sb")
            va_sb = const.tile([P, NQT * HL * VA], BF16, name="va_sb")
            yT_sb = const.tile([P, 2 * T], BF16, name="yT_sb")

            # ---- input DMAs: everything is a straight [128, X] image copy.
            # mask first (tiny) -- it feeds the PE warm-up matmuls below.
            # The prefix working set (wq + first x pieces + wk) is split
            # across FOUR issuing engines so it lands in parallel hardware
            # queues instead of serializing ~6MB behind one queue.
            nc.sync.dma_start(out=mk_sb[:, :], in_=mk[:, :])
            nc.sync.dma_start(out=wq_sb[:, :], in_=wqT[:, :])
            nc.scalar.dma_start(out=wk_sb[:, :], in_=wkT[:, :])
            for n in range(KC):
                eng = nc.sync if n % 2 == 0 else nc.scalar
                eng.dma_start(
                    out=xT_sb[:, n * T: n * T + 512],
                    in_=xT[:, n * T: n * T + 512],
                )
            nc.gpsimd.dma_start(out=wv_sb[:, :], in_=wvT[:, :])
            for n in range(KC):
                nc.gpsimd.dma_start(
                    out=xT_sb[:, n * T + 512: (n + 1) * T],
                    in_=xT[:, n * T + 512: (n + 1) * T],
                )
            nc.gpsimd.dma_start(out=wp_sb[:, :], in_=wpT[:, :])
            nc.vector.memset(ones_sb[:, :], 1.0)
            va_view = va_sb[:, :].rearrange("p (t h e) -> p t h e", t=NQT, h=HL)
            nc.vector.memset(va_view[:, :, :, D:VA], 1.0)

            # ---- merged QKV + attention + projection pipeline ----
            # The PE executes its queue IN ORDER, so emitting all of QKV
            # before attention serializes them (~63us of QKV before the
            # first exp).  Instead: a minimal QKV prefix, then the
            # remaining QKV tiles flow through a deadline-ordered work
            # queue sprinkled into the attention i-loops.  PSUM rings
            # (8 banks): s 2x2, y 2x1, aux 2x1 (qkv/proj/warmup/bcast).
            with tc.tile_pool(name="s_ps", bufs=2, space="PSUM") as s_pool, \
                 tc.tile_pool(name="y_ps", bufs=2, space="PSUM") as y_pool, \
                 tc.tile_pool(name="aux_ps", bufs=2, space="PSUM") as aux_pool, \
                 tc.tile_pool(name="dram_sc", bufs=1, space="DRAM") as dram_pool:
                den_dram = dram_pool.tile([NQC * HL, 512], F32, name="den_dram")
                rc_dram = dram_pool.tile([NQC * HL, 512], F32, name="rc_dram")

                # PE warm-up: the HAM clock gate only reaches 8/8 after
                # ~3.4us of sustained activity and the first real matmul
                # can't start until ~4us of input DMA has landed; burn the
                # window on throwaway matmuls over the first-loaded mask.
                wsc_sb = misc.tile([P, 512], BF16, name="wsc_sb", tag="wsc")
                nc.vector.memset(wsc_sb[:, :], 0.0)
                wps = aux_pool.tile([P, 512], F32, name="warmps", tag="aux")
                for w in range(7):
                    nc.tensor.matmul(
                        wps[:, :], mk_sb[:, :], wsc_sb[:, :],
                        start=True, stop=True,
                    )

                def warm_fill(n):
                    # throwaway matmuls that keep the HAM duty-cycle high
                    # across a known PE bubble (dependency-free, so they
                    # execute exactly when the queue would otherwise stall)
                    w_ps = aux_pool.tile([P, 512], F32, name="wfps", tag="aux")
                    for _ in range(n):
                        nc.tensor.matmul(
                            w_ps[:, :], mk_sb[:, :], wsc_sb[:, :],
                            start=True, stop=True,
                        )

                def qk_tile(w_sb, dst_sb, oc, tch):
                    ps = aux_pool.tile([P, 512], F32, name="qkps", tag="aux")
                    for kc in range(KC):
                        nc.tensor.matmul(
                            ps[:, :],
                            w_sb[:, kc * OL + oc * P: kc * OL + oc * P + P],
                            xT_sb[:, kc * T + tch * 512: kc * T + tch * 512 + 512],
                            start=(kc == 0),
                            stop=(kc == KC - 1),
                        )
                    # DVE eviction: ScalarE runs nothing but exp
                    nc.vector.tensor_copy(
                        dst_sb[:, oc * T + tch * 512: oc * T + tch * 512 + 512],
                        ps[:, :],
                    )

                def v_tile(tt):
                    ps = aux_pool.tile([P, 512], F32, name="vps", tag="aux")
                    for kc in range(KC):
                        nc.tensor.matmul(
                            ps[:, 0:OL],
                            xT_sb[:, kc * T + tt * P: kc * T + tt * P + P],
                            wv_sb[:, kc * OL:(kc + 1) * OL],
                            start=(kc == 0),
                            stop=(kc == KC - 1),
                        )
                    nc.vector.tensor_copy(
                        va_view[:, tt, :, 0:D],
                        ps[:, 0:OL].rearrange("p (h d) -> p h d", h=HL),
                    )

                def proj_tile(tile_idx, fine=False):
                    tt, ocn = divmod(tile_idx, 2)
                    trow = tt * P
                    pr_ps = aux_pool.tile([P, 512], F32, name="prps", tag="aux")
                    for cc in range(2):
                        nc.tensor.matmul(
                            pr_ps[:, :],
                            yT_sb[:, cc * T + trow: cc * T + trow + P],
                            wp_sb[:, cc * C + ocn * 512: cc * C + ocn * 512 + 512],
                            start=(cc == 0),
                            stop=(cc == 1),
                        )
                    o_sb = outp.tile([P, 512], F32, name="osb", tag="osb")
                    if not fine:
                        nc.vector.tensor_copy(o_sb[:, :], pr_ps[:, :])
                        # alternate DMA-issuing engine: descriptors land in
                        # two hardware queues, so the 256KB output tiles
                        # drain in parallel instead of serializing
                        if tile_idx % 2 == 0:
                            nc.sync.dma_start(
                                out=out[trow:trow + P, ocn * 512:(ocn + 1) * 512],
                                in_=o_sb[:, :],
                            )
                        else:
                            nc.scalar.dma_start(
                                out=out[trow:trow + P, ocn * 512:(ocn + 1) * 512],
                                in_=o_sb[:, :],
                            )
                    else:
                        # final burst: halve the eviction + DMA granularity
                        # and fan the halves across two queues so the last
                        # bytes leave as early as possible
                        for hf, eng in ((0, nc.sync), (1, nc.scalar)):
                            nc.vector.tensor_copy(
                                o_sb[:, hf * 256:(hf + 1) * 256],
                                pr_ps[:, hf * 256:(hf + 1) * 256],
                            )
                            eng.dma_start(
                                out=out[trow:trow + P,
                                        ocn * 512 + hf * 256: ocn * 512 + (hf + 1) * 256],
                                in_=o_sb[:, hf * 256:(hf + 1) * 256],
                            )

                # minimal prefix: exactly what attention chunk (0,0)'s first
                # S matmul needs (v tiles flow through the work queue -- the
                # first Y matmul only runs ~2 exps later)
                qk_tile(wq_sb, qT_sb, 0, 0)
                qk_tile(wk_sb, kT_sb, 0, 0)

                # the rest of QKV, deadline-ordered by the first chunk that
                # consumes each tile; popped one per attention iteration
                def mk_qk(w_sb, dst_sb, oc, tch):
                    return lambda: qk_tile(w_sb, dst_sb, oc, tch)

                def mk_v(tt):
                    return lambda: v_tile(tt)

                work = [mk_v(0), mk_v(1), mk_v(2), mk_v(3),
                        mk_qk(wq_sb, qT_sb, 1, 0), mk_qk(wk_sb, kT_sb, 1, 0)]
                for tch in (1, 2, 3):
                    work += [mk_qk(wq_sb, qT_sb, 0, tch),
                             mk_qk(wk_sb, kT_sb, 0, tch)]
                    work += [mk_v(tt) for tt in range(4 * tch, 4 * tch + 4)]
                    work += [mk_qk(wq_sb, qT_sb, 1, tch),
                             mk_qk(wk_sb, kT_sb, 1, tch)]
                # units that must be emitted before chunk (j4, hp) starts
                req = {(0, 0): 0, (0, 1): 6, (1, 0): 12, (1, 1): 14,
                       (2, 0): 20, (2, 1): 22, (3, 0): 28, (3, 1): 30}
                seq = [(a, b) for a in range(NQC) for b in range(2)]
                emitted = [0]

                def pop_work():
                    if work:
                        work.pop(0)()
                        emitted[0] += 1

                for j4 in range(NQC):
                    q0 = j4 * 512
                    for hp in range(2):
                        # flush any not-yet-emitted prerequisites
                        while emitted[0] < req[(j4, hp)]:
                            pop_work()
                        nxt = seq.index((j4, hp)) + 1
                        req_next = req[seq[nxt]] if nxt < len(seq) else 30
                        # previous chunk's projection tiles are sprinkled
                        # into the i-loop below: each proj MM is independent
                        # PE work that fills the S->exp->Y handoff bubble.
                        pend = (
                            [(j4 - 1) * 8 + hp * 4 + k for k in range(4)]
                            if j4 > 0 else []
                        )
                        # two heads interleaved per k-tile: one shared 2-bank
                        # S tile, one wide exp for both heads (the +352cyc
                        # ACTIVATE pipeline fill amortizes over 1024 cols),
                        # two independent y accumulations.  Doubles the
                        # PE-side work available per ACT op.
                        h0, h1 = 2 * hp, 2 * hp + 1
                        ch = hp
                        y0 = y_pool.tile([P, 512], F32, name="yps0", tag="yps")
                        y1 = y_pool.tile([P, 512], F32, name="yps1", tag="yps")
                        nk = 4 * (j4 + 1)

                        def emit_y(c0, p2, i):
                            for half, y_ps, hh in ((0, y0, h0), (1, y1, h1)):
                                nc.tensor.matmul(
                                    y_ps[0:VA, c0:512],
                                    va_sb[:, (i * HL + hh) * VA:(i * HL + hh) * VA + VA],
                                    p2[:, half * 512 + c0: half * 512 + 512],
                                    start=(i == 0),
                                    stop=(i == nk - 1),
                                )

                        prev_y = None
                        for i in range(nk):
                            m0 = max(0, i - 4 * j4)
                            c0 = P * m0
                            s2 = s_pool.tile([P, 1024], F32, name="sps", tag="sps")
                            for half, po in ((0, 0), (1, 64)):
                                nc.tensor.matmul(
                                    s2[:, half * 512 + c0: half * 512 + 512],
                                    kT_sb[po:po + D, ch * T + i * P: ch * T + i * P + P],
                                    qT_sb[po:po + D, ch * T + q0 + c0: ch * T + q0 + 512],
                                    start=True,
                                    stop=True,
                                )
                            p2 = att.tile([P, 1024], BF16, name="pt", tag="pt")
                            if m0 == 0:
                                nc.scalar.activation(
                                    p2[:, 0:1024], s2[:, 0:1024], Exp, scale=0.125
                                )
                            else:
                                # diagonal: the two live spans are disjoint;
                                # one 3D-AP exp covers both (halves the
                                # +352cyc ACTIVATE fills on the diagonal)
                                s2v = s2[:, :].rearrange("p (h c) -> p h c", h=2)
                                p2v = p2[:, :].rearrange("p (h c) -> p h c", h=2)
                                nc.scalar.activation(
                                    p2v[:, :, c0:512], s2v[:, :, c0:512],
                                    Exp, scale=0.125,
                                )
                            if i >= 4 * j4:
                                for half in range(2):
                                    nc.gpsimd.tensor_mul(
                                        p2[:, half * 512 + c0: half * 512 + c0 + P],
                                        p2[:, half * 512 + c0: half * 512 + c0 + P],
                                        mk_sb[:, :],
                                    )
                            # independent PE work between S(i) and Y(i-1):
                            # a QKV tile (paced so each chunk's inputs are
                            # ready one chunk ahead) or a proj tile.  These
                            # MMs execute while exp(i-1)/exp(i) run, so the
                            # in-order PE queue never parks on a Y waiting
                            # for its exp.
                            if emitted[0] < req_next:
                                pop_work()
                            elif pend and i % 2 == 1:
                                proj_tile(pend.pop(0))
                            elif work and i % 2 == 0:
                                pop_work()
                            if prev_y is not None:
                                emit_y(*prev_y)
                            prev_y = (c0, p2, i)
                        emit_y(*prev_y)
                        for t in pend:
                            proj_tile(t)

                        # tail: evict both heads' y (frees PSUM), push the two
                        # denominator rows to DRAM, pull them back spread over
                        # 8 partitions, one 8-lane iterative-divide reciprocal
                        # (~0.9us for both heads vs 3.3us/head single-lane),
                        # push back, broadcast-DMA, one DVE multiply per head.
                        # Everything between the eviction copy and the final
                        # multiply runs on DMA queues, off every engine.
                        if j4 == NQC - 1 and hp == 1:
                            # final chunk: the projection burst (and kernel
                            # end) gates on these yT rows -- take the
                            # shortest-latency path: single-lane reciprocal
                            # straight off the eviction copy, bf16 PE outer-
                            # product broadcast, one DVE multiply.  No DMA
                            # hops anywhere in the chain.  The warm-filler
                            # bridges the ~4us reciprocal chain so the
                            # projection burst runs at full clock.
                            warm_fill(20)
                            for half, y_ps, hh in ((0, y0, h0), (1, y1, h1)):
                                po = 64 * half
                                y_sb = misc.tile([P, 512], F32, name="ysb", tag="ysb")
                                nc.vector.tensor_copy(y_sb[0:VA, :], y_ps[0:VA, :])
                                rc = misc.tile([1, 512], F32, name="rc", tag="rc")
                                nc.vector.reciprocal(rc[:, :], y_sb[D:VA, :])
                                rcb = misc.tile([1, 512], BF16, name="rcb", tag="rcb")
                                nc.vector.tensor_copy(rcb[:, :], rc[:, :])
                                bc_ps = y_pool.tile([P, 512], F32, name="bcps", tag="yps")
                                nc.tensor.matmul(
                                    bc_ps[:, :], ones_sb[0:1, :], rcb[:, :],
                                    start=True, stop=True,
                                )
                                nc.vector.tensor_mul(
                                    yT_sb[po:po + D, ch * T + q0: ch * T + q0 + 512],
                                    y_sb[0:D, :],
                                    bc_ps[0:D, :],
                                )
                            for k in range(8):
                                proj_tile(j4 * 8 + k, fine=(k >= 4))
                            continue
                        pslot = 2 * (j4 * 2 + hp)
                        y_sbs = []
                        for half, y_ps, hh in ((0, y0, h0), (1, y1, h1)):
                            y_sb = misc.tile([P, 512], F32, name="ysb", tag="ysb")
                            nc.vector.tensor_copy(y_sb[0:VA, :], y_ps[0:VA, :])
                            nc.sync.dma_start(
                                out=den_dram[pslot + half: pslot + half + 1, :],
                                in_=y_sb[D:VA, :],
                            )
                            y_sbs.append(y_sb)
                        dsp = misc.tile([8, 128], F32, name="dsp", tag="dsp")
                        nc.sync.dma_start(
                            out=dsp[:, :],
                            in_=den_dram[pslot: pslot + 2, :].rearrange(
                                "a (p b) -> (a p) b", p=4
                            ),
                        )
                        rc8 = misc.tile([8, 128], F32, name="rc8", tag="rc8")
                        nc.vector.reciprocal(rc8[:, :], dsp[:, :])
                        nc.sync.dma_start(
                            out=rc_dram[pslot: pslot + 2, :].rearrange(
                                "a (p b) -> (a p) b", p=4
                            ),
                            in_=rc8[:, :],
                        )
                        for half, hh in ((0, h0), (1, h1)):
                            po = 64 * half
                            bc_sb = misc.tile([P, 512], F32, name="bcsb", tag="bcsb")
                            nc.sync.dma_start(
                                out=bc_sb[:, :],
                                in_=rc_dram[pslot + half: pslot + half + 1, :]
                                .to_broadcast((P, 512)),
                            )
                            nc.vector.tensor_mul(
                                yT_sb[po:po + D, ch * T + q0: ch * T + q0 + 512],
                                y_sbs[half][0:D, :],
                                bc_sb[0:D, :],
                            )
    nc.compile()
    return nc


def _get_nc():
    global _nc_cache
    if _nc_cache is None:
        _nc_cache = _build_nc()
    return _nc_cache


def _prepare_in_maps(x, W_qkv, W_proj):
    x = np.asarray(x, np.float32)
    W_qkv = np.asarray(W_qkv, np.float32)
    W_proj = np.asarray(W_proj, np.float32)
    # [r, j] = 1 where j >= r (upper triangle incl diag, in S^T [k, q] layout)
    tri = (np.arange(P)[None, :] >= np.arange(P)[:, None]).astype(np.float32)
    tri = tri.astype(_BF16)
    in_maps = []
    for c in range(8):
        b, g = c // 4, c % 4
        r0 = OL * g
        def img(a):
            # [R, Y] with R = n*128 rows -> SBUF image [128, n*Y]
            n = a.shape[0] // P
            return np.ascontiguousarray(
                a.reshape(n, P, a.shape[1]).transpose(1, 0, 2).reshape(P, -1)
            ).astype(_BF16)

        in_maps.append({
            "xT": img(x[b].T),
            "wqT": img(W_qkv[r0:r0 + OL, :].T),
            "wkT": img(W_qkv[C + r0:C + r0 + OL, :].T),
            "wvT": img(W_qkv[2 * C + r0:2 * C + r0 + OL, :].T),
            "wpT": img(W_proj[:, r0:r0 + OL].T),
            "mask_tri": tri,
        })
    return in_maps


def _combine(results):
    out = np.zeros((B, T, C), np.float32)
    for c in range(8):
        out[c // 4] += results[c]["out"]
    return out


def kernel(x, W_qkv, W_proj):
    nc = _get_nc()
    in_maps = _prepare_in_maps(x, W_qkv, W_proj)
    try:
        res = bass_utils.run_bass_kernel_spmd(nc, in_maps, core_ids=list(range(8)))
    except Exception:
        # rare transient NRT device errors; one retry
        res = bass_utils.run_bass_kernel_spmd(nc, in_maps, core_ids=list(range(8)))
    return _combine(res.results)


def kernel_traced(x, W_qkv, W_proj, trace_cores=None):
    """Like kernel() but returns (out, exec_time_ns) using an NTFF profile."""
    nc = _get_nc()
    in_maps = _prepare_in_maps(x, W_qkv, W_proj)
    res = bass_utils.run_bass_kernel_spmd(
        nc, in_maps, core_ids=list(range(8)), trace=True, trace_cores=trace_cores
    )
    return _combine(res.results), res.exec_time_ns



# revision 53
# speedup vs baseline: 1.0240x; 1.0240x over previous
"""Trainium2 Bass kernel: causal self-attention (B=2, T=2048, C=1024, H=16, Dh=64).

Sharding: 8 cores = 2 (batch) x 4 (head groups of 4 heads).  Each core gets
x[b] plus the W_qkv rows / W_proj columns for its heads, computes the full
attention + a partial output projection for its batch, and the host sums the
4 partials per batch (tensor-parallel unshard).

All matmuls run in bf16 with f32 PSUM accumulation.  x is passed transposed
(xT = x[b].T) so that:
  qT, kT = Wq @ xT, Wk @ xT     (head dim on partitions)  -- no transposes
  v      = xT.T @ WvT           (natural [T, d] layout)
  S^T    = kT_h(tile).T @ qT_h  ([k, q] layout, 128x512 blocks, the two
           heads of a pair row-tiled into array halves -> concurrent)
  exp on ScalarE (logits are bounded, no max pass needed); causal masking by
  computing only the live columns of each block plus one multiplicative
  [128,128] triangle mask on the diagonal subtile (gpsimd); row sums via a
  ones column appended to V (P@[V|1] accumulates y^T and the softmax
  denominators in one PSUM tile).
  out_partial = y^T.T @ WpT   (f32, DMA'd out).

Schedule: the PE executes its queue IN ORDER, so QKV tiles are emitted
through a deadline-ordered work queue sprinkled one-per-iteration into the
attention i-loops (plus previous-chunk projection tiles), never as a bulk
phase.  This keeps the PE continuously busy from ~10us on: the S->exp->Y
handoff bubbles are filled with independent QKV/projection matmuls, which
also keeps the HAM activity monitor at K=8/8 (full 2.4GHz clock) instead of
oscillating into the 1.2GHz throttle state.  Y matmuls are emitted one
iteration behind their exp (software pipelining) so the in-order PE queue
never parks on an unfinished ACTIVATE.

Softmax normalization: denominator rows go through a DRAM round-trip that
re-spreads the 512 q-columns over 8 SBUF partitions, so the iterative-
divide DVE RECIPROCAL runs 8 lanes wide (0.9us for two heads vs 3.3us per
head single-lane); the reciprocal row is broadcast back over 128 partitions
by a stride-0 DRAM-read DMA and applied with one DVE multiply.  The final
chunk's tail gates the last projection burst (and the kernel end), so
throwaway warm-filler matmuls bridge its DMA-chain latency -- the burst
then runs at full clock -- and the burst's last tiles split their output
DMAs across two hardware queues at 256-column granularity.

ScalarE runs nothing but Exp (plus two DMA descriptor issues per projection
pair); every PSUM eviction lives on the DVE.  Diagonal-block exps cover
both disjoint live spans with one 3D-AP ACTIVATE, halving the +352-cycle
pipeline-fill cost on the diagonal.
"""
import sys
import types

import numpy as np
import ml_dtypes

_BF16 = ml_dtypes.bfloat16


def _install_ntff_hook():
    """Provide antenv.axon_hooks so run_bass_kernel_spmd(trace=True) works."""
    if "antenv.axon_hooks" in sys.modules:
        return
    mod = types.ModuleType("antenv.axon_hooks")
    mod._hook = None

    def set_axon_ntff_profile_hook(h):
        mod._hook = h

    def get_axon_ntff_profile_hook():
        return mod._hook

    mod.set_axon_ntff_profile_hook = set_axon_ntff_profile_hook
    mod.get_axon_ntff_profile_hook = get_axon_ntff_profile_hook
    sys.modules["antenv.axon_hooks"] = mod
    try:
        import antenv

        antenv.axon_hooks = mod
    except Exception:
        pass
    try:
        from trn_agent_boot.trn_boot import _ntff_profile_via_ctypes

        mod.set_axon_ntff_profile_hook(
            _ntff_profile_via_ctypes("/opt/axon/libaxon_pjrt.so")
        )
    except Exception:
        pass


_install_ntff_hook()

import concourse.bacc as bacc
import concourse.mybir as mybir
from concourse import bass_utils
from concourse.tile import TileContext

# no network bucket in this container; keep artifacts local
bass_utils.upload_artifacts = lambda tmpdir: tmpdir

BF16 = mybir.dt.bfloat16
F32 = mybir.dt.float32

B, T, C = 2, 2048, 1024
H, D = 16, 64
HL = 4            # heads per core
OL = HL * D       # 256 local qkv output dim
P = 128
KC = C // P       # 8 contraction chunks
NQT = T // P      # 16 q/k 128-tiles
NQC = T // 512    # 4 q 512-chunks
VA = D + 1        # v columns per head incl. ones column (65)

_nc_cache = None


def _build_nc():
    nc = bacc.Bacc("TRN2", target_bir_lowering=False, debug=False, num_devices=8)

    # all inputs arrive pre-arranged in SBUF-image layout [128, X] so every
    # input DMA moves multi-KB contiguous runs per partition row.
    xT = nc.declare_dram_parameter("xT", [P, KC * T], BF16, isOutput=False)
    wqT = nc.declare_dram_parameter("wqT", [P, KC * OL], BF16, isOutput=False)
    wkT = nc.declare_dram_parameter("wkT", [P, KC * OL], BF16, isOutput=False)
    wvT = nc.declare_dram_parameter("wvT", [P, KC * OL], BF16, isOutput=False)
    wpT = nc.declare_dram_parameter("wpT", [P, 2 * C], BF16, isOutput=False)
    mk = nc.declare_dram_parameter("mask_tri", [P, P], BF16, isOutput=False)
    out = nc.declare_dram_parameter("out", [T, C], F32, isOutput=True)

    Exp = mybir.ActivationFunctionType.Exp

    with TileContext(nc) as tc:
        with tc.tile_pool(name="const", bufs=1) as const, \
             tc.tile_pool(name="misc", bufs=3) as misc, \
             tc.tile_pool(name="att", bufs=8) as att, \
             tc.tile_pool(name="outp", bufs=6) as outp:
            xT_sb = const.tile([P, KC * T], BF16, name="xT_sb")
            wq_sb = const.tile([P, KC * OL], BF16, name="wq_sb")
            wk_sb = const.tile([P, KC * OL], BF16, name="wk_sb")
            wv_sb = const.tile([P, KC * OL], BF16, name="wv_sb")
            wp_sb = const.tile([P, 2 * C], BF16, name="wp_sb")
            mk_sb = const.tile([P, P], BF16, name="mk_sb")
            qT_sb = const.tile([P, 2 * T], BF16, name="qT_sb")
            kT_sb = const.tile([P, 2 * T], BF16, name="kT_sb")
            va_sb = const.tile([P, NQT * HL * VA], BF16, name="va_sb")
            yT_sb = const.tile([P, 2 * T], BF16, name="yT_sb")

            # ---- input DMAs: everything is a straight [128, X] image copy.
            # mask first (tiny) -- it feeds the PE warm-up matmuls below.
            # The prefix working set (wq + first x pieces + wk) is split
            # across FOUR issuing engines so it lands in parallel hardware
            # queues instead of serializing ~6MB behind one queue.
            nc.sync.dma_start(out=mk_sb[:, :], in_=mk[:, :])
            nc.sync.dma_start(out=wq_sb[:, :], in_=wqT[:, :])
            for n in range(KC):
                nc.sync.dma_start(
                    out=xT_sb[:, n * T: n * T + 512],
                    in_=xT[:, n * T: n * T + 512],
                )
            nc.sync.dma_start(out=wk_sb[:, :], in_=wkT[:, :])
            nc.sync.dma_start(out=wv_sb[:, :], in_=wvT[:, :])
            for n in range(KC):
                nc.sync.dma_start(
                    out=xT_sb[:, n * T + 512: (n + 1) * T],
                    in_=xT[:, n * T + 512: (n + 1) * T],
                )
            nc.sync.dma_start(out=wp_sb[:, :], in_=wpT[:, :])
            va_view = va_sb[:, :].rearrange("p (t h e) -> p t h e", t=NQT, h=HL)
            nc.vector.memset(va_view[:, :, :, D:VA], 1.0)

            # ---- merged QKV + attention + projection pipeline ----
            # The PE executes its queue IN ORDER, so emitting all of QKV
            # before attention serializes them (~63us of QKV before the
            # first exp).  Instead: a minimal QKV prefix, then the
            # remaining QKV tiles flow through a deadline-ordered work
            # queue sprinkled into the attention i-loops.  PSUM rings
            # (8 banks): s 2x2, y 2x1, aux 2x1 (qkv/proj/warmup/bcast).
            with tc.tile_pool(name="s_ps", bufs=2, space="PSUM") as s_pool, \
                 tc.tile_pool(name="y_ps", bufs=2, space="PSUM") as y_pool, \
                 tc.tile_pool(name="aux_ps", bufs=2, space="PSUM") as aux_pool, \
                 tc.tile_pool(name="dram_sc", bufs=1, space="DRAM") as dram_pool:
                den_dram = dram_pool.tile([NQC * HL, 512], F32, name="den_dram")
                rc_dram = dram_pool.tile([NQC * HL, 512], F32, name="rc_dram")

                # PE warm-up: the HAM clock gate only reaches 8/8 after
                # ~3.4us of sustained activity and the first real matmul
                # can't start until ~4us of input DMA has landed; burn the
                # window on throwaway matmuls over the first-loaded mask.
                wsc_sb = misc.tile([P, 512], BF16, name="wsc_sb", tag="wsc")
                nc.vector.memset(wsc_sb[:, :], 0.0)
                wps = aux_pool.tile([P, 512], F32, name="warmps", tag="aux")
                for w in range(7):
                    nc.tensor.matmul(
                        wps[:, :], mk_sb[:, :], wsc_sb[:, :],
                        start=True, stop=True,
                    )

                def warm_fill(n):
                    # throwaway matmuls that keep the HAM duty-cycle high
                    # across a known PE bubble (dependency-free, so they
                    # execute exactly when the queue would otherwise stall)
                    w_ps = aux_pool.tile([P, 512], F32, name="wfps", tag="aux")
                    for _ in range(n):
                        nc.tensor.matmul(
                            w_ps[:, :], mk_sb[:, :], wsc_sb[:, :],
                            start=True, stop=True,
                        )

                def qk_tile(w_sb, dst_sb, oc, tch):
                    ps = aux_pool.tile([P, 512], F32, name="qkps", tag="aux")
                    for kc in range(KC):
                        nc.tensor.matmul(
                            ps[:, :],
                            w_sb[:, kc * OL + oc * P: kc * OL + oc * P + P],
                            xT_sb[:, kc * T + tch * 512: kc * T + tch * 512 + 512],
                            start=(kc == 0),
                            stop=(kc == KC - 1),
                        )
                    # DVE eviction: ScalarE runs nothing but exp
                    nc.vector.tensor_copy(
                        dst_sb[:, oc * T + tch * 512: oc * T + tch * 512 + 512],
                        ps[:, :],
                    )

                def v_tile(tt):
                    ps = aux_pool.tile([P, 512], F32, name="vps", tag="aux")
                    for kc in range(KC):
                        nc.tensor.matmul(
                            ps[:, 0:OL],
                            xT_sb[:, kc * T + tt * P: kc * T + tt * P + P],
                            wv_sb[:, kc * OL:(kc + 1) * OL],
                            start=(kc == 0),
                            stop=(kc == KC - 1),
                        )
                    nc.vector.tensor_copy(
                        va_view[:, tt, :, 0:D],
                        ps[:, 0:OL].rearrange("p (h d) -> p h d", h=HL),
                    )

                def proj_tile(tile_idx, fine=False):
                    tt, ocn = divmod(tile_idx, 2)
                    trow = tt * P
                    pr_ps = aux_pool.tile([P, 512], F32, name="prps", tag="aux")
                    for cc in range(2):
                        nc.tensor.matmul(
                            pr_ps[:, :],
                            yT_sb[:, cc * T + trow: cc * T + trow + P],
                            wp_sb[:, cc * C + ocn * 512: cc * C + ocn * 512 + 512],
                            start=(cc == 0),
                            stop=(cc == 1),
                        )
                    o_sb = outp.tile([P, 512], F32, name="osb", tag="osb")
                    if not fine:
                        nc.vector.tensor_copy(o_sb[:, :], pr_ps[:, :])
                        # alternate DMA-issuing engine: descriptors land in
                        # two hardware queues, so the 256KB output tiles
                        # drain in parallel instead of serializing
                        if tile_idx % 2 == 0:
                            nc.sync.dma_start(
                                out=out[trow:trow + P, ocn * 512:(ocn + 1) * 512],
                                in_=o_sb[:, :],
                            )
                        else:
                            nc.scalar.dma_start(
                                out=out[trow:trow + P, ocn * 512:(ocn + 1) * 512],
                                in_=o_sb[:, :],
                            )
                    else:
                        # final burst: halve the eviction + DMA granularity
                        # and fan the halves across two queues so the last
                        # bytes leave as early as possible
                        for hf, eng in ((0, nc.sync), (1, nc.scalar)):
                            nc.vector.tensor_copy(
                                o_sb[:, hf * 256:(hf + 1) * 256],
                                pr_ps[:, hf * 256:(hf + 1) * 256],
                            )
                            eng.dma_start(
                                out=out[trow:trow + P,
                                        ocn * 512 + hf * 256: ocn * 512 + (hf + 1) * 256],
                                in_=o_sb[:, hf * 256:(hf + 1) * 256],
                            )

                # minimal prefix: exactly what attention chunk (0,0)'s first
                # S matmul needs (v tiles flow through the work queue -- the
                # first Y matmul only runs ~2 exps later)
                qk_tile(wq_sb, qT_sb, 0, 0)
                qk_tile(wk_sb, kT_sb, 0, 0)

                # the rest of QKV, deadline-ordered by the first chunk that
                # consumes each tile; popped one per attention iteration
                def mk_qk(w_sb, dst_sb, oc, tch):
                    return lambda: qk_tile(w_sb, dst_sb, oc, tch)

                def mk_v(tt):
                    return lambda: v_tile(tt)

                work = [mk_v(0), mk_v(1), mk_v(2), mk_v(3),
                        mk_qk(wq_sb, qT_sb, 1, 0), mk_qk(wk_sb, kT_sb, 1, 0)]
                for tch in (1, 2, 3):
                    work += [mk_qk(wq_sb, qT_sb, 0, tch),
                             mk_qk(wk_sb, kT_sb, 0, tch)]
                    work += [mk_v(tt) for tt in range(4 * tch, 4 * tch + 4)]
                    work += [mk_qk(wq_sb, qT_sb, 1, tch),
                             mk_qk(wk_sb, kT_sb, 1, tch)]
                # units that must be emitted before chunk (j4, hp) starts
                req = {(0, 0): 0, (0, 1): 6, (1, 0): 12, (1, 1): 14,
                       (2, 0): 20, (2, 1): 22, (3, 0): 28, (3, 1): 30}
                seq = [(a, b) for a in range(NQC) for b in range(2)]
                emitted = [0]

                def pop_work():
                    if work:
                        work.pop(0)()
                        emitted[0] += 1

                for j4 in range(NQC):
                    q0 = j4 * 512
                    for hp in range(2):
                        # flush any not-yet-emitted prerequisites
                        while emitted[0] < req[(j4, hp)]:
                            pop_work()
                        nxt = seq.index((j4, hp)) + 1
                        req_next = req[seq[nxt]] if nxt < len(seq) else 30
                        # previous chunk's projection tiles are sprinkled
                        # into the i-loop below: each proj MM is independent
                        # PE work that fills the S->exp->Y handoff bubble.
                        pend = (
                            [(j4 - 1) * 8 + hp * 4 + k for k in range(4)]
                            if j4 > 0 else []
                        )
                        # two heads interleaved per k-tile: one shared 2-bank
                        # S tile, one wide exp for both heads (the +352cyc
                        # ACTIVATE pipeline fill amortizes over 1024 cols),
                        # two independent y accumulations.  Doubles the
                        # PE-side work available per ACT op.
                        h0, h1 = 2 * hp, 2 * hp + 1
                        ch = hp
                        y0 = y_pool.tile([P, 512], F32, name="yps0", tag="yps")
                        y1 = y_pool.tile([P, 512], F32, name="yps1", tag="yps")
                        nk = 4 * (j4 + 1)

                        def emit_y(c0, p2, i):
                            for half, y_ps, hh in ((0, y0, h0), (1, y1, h1)):
                                nc.tensor.matmul(
                                    y_ps[0:VA, c0:512],
                                    va_sb[:, (i * HL + hh) * VA:(i * HL + hh) * VA + VA],
                                    p2[:, half * 512 + c0: half * 512 + 512],
                                    start=(i == 0),
                                    stop=(i == nk - 1),
                                )

                        prev_y = None
                        for i in range(nk):
                            m0 = max(0, i - 4 * j4)
                            c0 = P * m0
                            s2 = s_pool.tile([P, 1024], F32, name="sps", tag="sps")
                            for half, po in ((0, 0), (1, 64)):
                                nc.tensor.matmul(
                                    s2[:, half * 512 + c0: half * 512 + 512],
                                    kT_sb[po:po + D, ch * T + i * P: ch * T + i * P + P],
                                    qT_sb[po:po + D, ch * T + q0 + c0: ch * T + q0 + 512],
                                    start=True,
                                    stop=True,
                                )
                            p2 = att.tile([P, 1024], BF16, name="pt", tag="pt")
                            if m0 == 0:
                                nc.scalar.activation(
                                    p2[:, 0:1024], s2[:, 0:1024], Exp, scale=0.125
                                )
                            else:
                                # diagonal: the two live spans are disjoint;
                                # one 3D-AP exp covers both (halves the
                                # +352cyc ACTIVATE fills on the diagonal)
                                s2v = s2[:, :].rearrange("p (h c) -> p h c", h=2)
                                p2v = p2[:, :].rearrange("p (h c) -> p h c", h=2)
                                nc.scalar.activation(
                                    p2v[:, :, c0:512], s2v[:, :, c0:512],
                                    Exp, scale=0.125,
                                )
                            if i >= 4 * j4:
                                for half in range(2):
                                    nc.gpsimd.tensor_mul(
                                        p2[:, half * 512 + c0: half * 512 + c0 + P],
                                        p2[:, half * 512 + c0: half * 512 + c0 + P],
                                        mk_sb[:, :],
                                    )
                            # independent PE work between S(i) and Y(i-1):
                            # a QKV tile (paced so each chunk's inputs are
                            # ready one chunk ahead) or a proj tile.  These
                            # MMs execute while exp(i-1)/exp(i) run, so the
                            # in-order PE queue never parks on a Y waiting
                            # for its exp.
                            if emitted[0] < req_next:
                                pop_work()
                            elif pend and i % 2 == 1:
                                proj_tile(pend.pop(0))
                            elif work and i % 2 == 0:
                                pop_work()
                            if prev_y is not None:
                                emit_y(*prev_y)
                            prev_y = (c0, p2, i)
                        emit_y(*prev_y)
                        for t in pend:
                            proj_tile(t)

                        # tail: evict both heads' y (frees PSUM), push the two
                        # denominator rows to DRAM, pull them back spread over
                        # 8 partitions, one 8-lane iterative-divide reciprocal
                        # (~0.9us for both heads vs 3.3us/head single-lane),
                        # push back, broadcast-DMA, one DVE multiply per head.
                        # Everything between the eviction copy and the final
                        # multiply runs on DMA queues, off every engine.
                        # final chunk: the projection burst (and kernel end)
                        # gate on this tail's yT rows; warm-filler matmuls
                        # bridge the tail's DMA-chain latency so the burst
                        # runs at full clock.
                        if j4 == NQC - 1 and hp == 1:
                            warm_fill(26)
                        pslot = 2 * (j4 * 2 + hp)
                        y_sbs = []
                        for half, y_ps, hh in ((0, y0, h0), (1, y1, h1)):
                            y_sb = misc.tile([P, 512], F32, name="ysb", tag="ysb")
                            nc.vector.tensor_copy(y_sb[0:VA, :], y_ps[0:VA, :])
                            nc.sync.dma_start(
                                out=den_dram[pslot + half: pslot + half + 1, :],
                                in_=y_sb[D:VA, :],
                            )
                            y_sbs.append(y_sb)
                        dsp = misc.tile([8, 128], F32, name="dsp", tag="dsp")
                        nc.sync.dma_start(
                            out=dsp[:, :],
                            in_=den_dram[pslot: pslot + 2, :].rearrange(
                                "a (p b) -> (a p) b", p=4
                            ),
                        )
                        rc8 = misc.tile([8, 128], F32, name="rc8", tag="rc8")
                        nc.vector.reciprocal(rc8[:, :], dsp[:, :])
                        nc.sync.dma_start(
                            out=rc_dram[pslot: pslot + 2, :].rearrange(
                                "a (p b) -> (a p) b", p=4
                            ),
                            in_=rc8[:, :],
                        )
                        for half, hh in ((0, h0), (1, h1)):
                            po = 64 * half
                            bc_sb = misc.tile([P, 512], F32, name="bcsb", tag="bcsb")
                            nc.sync.dma_start(
                                out=bc_sb[:, :],
                                in_=rc_dram[pslot + half: pslot + half + 1, :]
                                .to_broadcast((P, 512)),
                            )
                            nc.vector.tensor_mul(
                                yT_sb[po:po + D, ch * T + q0: ch * T + q0 + 512],
                                y_sbs[half][0:D, :],
                                bc_sb[0:D, :],
                            )
                        if j4 == NQC - 1 and hp == 1:
                            for k in range(8):
                                proj_tile(j4 * 8 + k, fine=(k >= 4))
    nc.compile()
    return nc


def _get_nc():
    global _nc_cache
    if _nc_cache is None:
        _nc_cache = _build_nc()
    return _nc_cache


def _prepare_in_maps(x, W_qkv, W_proj):
    x = np.asarray(x, np.float32)
    W_qkv = np.asarray(W_qkv, np.float32)
    W_proj = np.asarray(W_proj, np.float32)
    # [r, j] = 1 where j >= r (upper triangle incl diag, in S^T [k, q] layout)
    tri = (np.arange(P)[None, :] >= np.arange(P)[:, None]).astype(np.float32)
    tri = tri.astype(_BF16)
    in_maps = []
    for c in range(8):
        b, g = c // 4, c % 4
        r0 = OL * g
        def img(a):
            # [R, Y] with R = n*128 rows -> SBUF image [128, n*Y]
            n = a.shape[0] // P
            return np.ascontiguousarray(
                a.reshape(n, P, a.shape[1]).transpose(1, 0, 2).reshape(P, -1)
            ).astype(_BF16)

        in_maps.append({
            "xT": img(x[b].T),
            "wqT": img(W_qkv[r0:r0 + OL, :].T),
            "wkT": img(W_qkv[C + r0:C + r0 + OL, :].T),
            "wvT": img(W_qkv[2 * C + r0:2 * C + r0 + OL, :].T),
            "wpT": img(W_proj[:, r0:r0 + OL].T),
            "mask_tri": tri,
        })
    return in_maps


def _combine(results):
    out = np.zeros((B, T, C), np.float32)
    for c in range(8):
        out[c // 4] += results[c]["out"]
    return out


def kernel(x, W_qkv, W_proj):
    nc = _get_nc()
    in_maps = _prepare_in_maps(x, W_qkv, W_proj)
    try:
        res = bass_utils.run_bass_kernel_spmd(nc, in_maps, core_ids=list(range(8)))
    except Exception:
        # rare transient NRT device errors; one retry
        res = bass_utils.run_bass_kernel_spmd(nc, in_maps, core_ids=list(range(8)))
    return _combine(res.results)


def kernel_traced(x, W_qkv, W_proj, trace_cores=None):
    """Like kernel() but returns (out, exec_time_ns) using an NTFF profile."""
    nc = _get_nc()
    in_maps = _prepare_in_maps(x, W_qkv, W_proj)
    res = bass_utils.run_bass_kernel_spmd(
        nc, in_maps, core_ids=list(range(8)), trace=True, trace_cores=trace_cores
    )
    return _combine(res.results), res.exec_time_ns



# revision 54
# speedup vs baseline: 1.0253x; 1.0012x over previous
"""Trainium2 Bass kernel: causal self-attention (B=2, T=2048, C=1024, H=16, Dh=64).

Sharding: 8 cores = 2 (batch) x 4 (head groups of 4 heads).  Each core gets
x[b] plus the W_qkv rows / W_proj columns for its heads, computes the full
attention + a partial output projection for its batch, and the host sums the
4 partials per batch (tensor-parallel unshard).

All matmuls run in bf16 with f32 PSUM accumulation.  x is passed transposed
(xT = x[b].T) so that:
  qT, kT = Wq @ xT, Wk @ xT     (head dim on partitions)  -- no transposes
  v      = xT.T @ WvT           (natural [T, d] layout)
  S^T    = kT_h(tile).T @ qT_h  ([k, q] layout, 128x512 blocks, the two
           heads of a pair row-tiled into array halves -> concurrent)
  exp on ScalarE (logits are bounded, no max pass needed); causal masking by
  computing only the live columns of each block plus one multiplicative
  [128,128] triangle mask on the diagonal subtile (gpsimd); row sums via a
  ones column appended to V (P@[V|1] accumulates y^T and the softmax
  denominators in one PSUM tile).
  out_partial = y^T.T @ WpT   (f32, DMA'd out).

Schedule: the PE executes its queue IN ORDER, so QKV tiles are emitted
through a deadline-ordered work queue sprinkled one-per-iteration into the
attention i-loops (plus previous-chunk projection tiles), never as a bulk
phase.  This keeps the PE continuously busy from ~10us on: the S->exp->Y
handoff bubbles are filled with independent QKV/projection matmuls, which
also keeps the HAM activity monitor at K=8/8 (full 2.4GHz clock) instead of
oscillating into the 1.2GHz throttle state.  Y matmuls are emitted one
iteration behind their exp (software pipelining) so the in-order PE queue
never parks on an unfinished ACTIVATE.

Softmax normalization: denominator rows go through a DRAM round-trip that
re-spreads the 512 q-columns over 8 SBUF partitions, so the iterative-
divide DVE RECIPROCAL runs 8 lanes wide (0.9us for two heads vs 3.3us per
head single-lane); the reciprocal row is broadcast back over 128 partitions
by a stride-0 DRAM-read DMA and applied with one DVE multiply.  The final
chunk's tail gates the last projection burst (and the kernel end), so
throwaway warm-filler matmuls bridge its DMA-chain latency -- the burst
then runs at full clock -- and the burst's last tiles split their output
DMAs across two hardware queues at 256-column granularity.

ScalarE runs nothing but Exp (plus two DMA descriptor issues per projection
pair); every PSUM eviction lives on the DVE.  Diagonal-block exps cover
both disjoint live spans with one 3D-AP ACTIVATE, halving the +352-cycle
pipeline-fill cost on the diagonal.
"""
import sys
import types

import numpy as np
import ml_dtypes

_BF16 = ml_dtypes.bfloat16


def _install_ntff_hook():
    """Provide antenv.axon_hooks so run_bass_kernel_spmd(trace=True) works."""
    if "antenv.axon_hooks" in sys.modules:
        return
    mod = types.ModuleType("antenv.axon_hooks")
    mod._hook = None

    def set_axon_ntff_profile_hook(h):
        mod._hook = h

    def get_axon_ntff_profile_hook():
        return mod._hook

    mod.set_axon_ntff_profile_hook = set_axon_ntff_profile_hook
    mod.get_axon_ntff_profile_hook = get_axon_ntff_profile_hook
    sys.modules["antenv.axon_hooks"] = mod
    try:
        import antenv

        antenv.axon_hooks = mod
    except Exception:
        pass
    try:
        from trn_agent_boot.trn_boot import _ntff_profile_via_ctypes

        mod.set_axon_ntff_profile_hook(
            _ntff_profile_via_ctypes("/opt/axon/libaxon_pjrt.so")
        )
    except Exception:
        pass


_install_ntff_hook()

import concourse.bacc as bacc
import concourse.mybir as mybir
from concourse import bass_utils
from concourse.tile import TileContext

# no network bucket in this container; keep artifacts local
bass_utils.upload_artifacts = lambda tmpdir: tmpdir

BF16 = mybir.dt.bfloat16
F32 = mybir.dt.float32

B, T, C = 2, 2048, 1024
H, D = 16, 64
HL = 4            # heads per core
OL = HL * D       # 256 local qkv output dim
P = 128
KC = C // P       # 8 contraction chunks
NQT = T // P      # 16 q/k 128-tiles
NQC = T // 512    # 4 q 512-chunks
VA = D + 1        # v columns per head incl. ones column (65)

_nc_cache = None


def _build_nc():
    nc = bacc.Bacc("TRN2", target_bir_lowering=False, debug=False, num_devices=8)

    # all inputs arrive pre-arranged in SBUF-image layout [128, X] so every
    # input DMA moves multi-KB contiguous runs per partition row.
    xT = nc.declare_dram_parameter("xT", [P, KC * T], BF16, isOutput=False)
    wqT = nc.declare_dram_parameter("wqT", [P, KC * OL], BF16, isOutput=False)
    wkT = nc.declare_dram_parameter("wkT", [P, KC * OL], BF16, isOutput=False)
    wvT = nc.declare_dram_parameter("wvT", [P, KC * OL], BF16, isOutput=False)
    wpT = nc.declare_dram_parameter("wpT", [P, 2 * C], BF16, isOutput=False)
    mk = nc.declare_dram_parameter("mask_tri", [P, P], BF16, isOutput=False)
    out = nc.declare_dram_parameter("out", [T, C], F32, isOutput=True)

    Exp = mybir.ActivationFunctionType.Exp

    with TileContext(nc) as tc:
        with tc.tile_pool(name="const", bufs=1) as const, \
             tc.tile_pool(name="misc", bufs=3) as misc, \
             tc.tile_pool(name="att", bufs=8) as att, \
             tc.tile_pool(name="outp", bufs=6) as outp:
            xT_sb = const.tile([P, KC * T], BF16, name="xT_sb")
            wq_sb = const.tile([P, KC * OL], BF16, name="wq_sb")
            wk_sb = const.tile([P, KC * OL], BF16, name="wk_sb")
            wv_sb = const.tile([P, KC * OL], BF16, name="wv_sb")
            wp_sb = const.tile([P, 2 * C], BF16, name="wp_sb")
            mk_sb = const.tile([P, P], BF16, name="mk_sb")
            qT_sb = const.tile([P, 2 * T], BF16, name="qT_sb")
            kT_sb = const.tile([P, 2 * T], BF16, name="kT_sb")
            va_sb = const.tile([P, NQT * HL * VA], BF16, name="va_sb")
            yT_sb = const.tile([P, 2 * T], BF16, name="yT_sb")

            # ---- input DMAs: everything is a straight [128, X] image copy.
            # mask first (tiny) -- it feeds the PE warm-up matmuls below.
            # The prefix working set (wq + first x pieces + wk) is split
            # across FOUR issuing engines so it lands in parallel hardware
            # queues instead of serializing ~6MB behind one queue.
            nc.sync.dma_start(out=mk_sb[:, :], in_=mk[:, :])
            nc.sync.dma_start(out=wq_sb[:, :], in_=wqT[:, :])
            for n in range(KC):
                nc.sync.dma_start(
                    out=xT_sb[:, n * T: n * T + 512],
                    in_=xT[:, n * T: n * T + 512],
                )
            nc.sync.dma_start(out=wk_sb[:, :], in_=wkT[:, :])
            nc.sync.dma_start(out=wv_sb[:, :], in_=wvT[:, :])
            for n in range(KC):
                nc.sync.dma_start(
                    out=xT_sb[:, n * T + 512: (n + 1) * T],
                    in_=xT[:, n * T + 512: (n + 1) * T],
                )
            nc.sync.dma_start(out=wp_sb[:, :], in_=wpT[:, :])
            va_view = va_sb[:, :].rearrange("p (t h e) -> p t h e", t=NQT, h=HL)
            nc.vector.memset(va_view[:, :, :, D:VA], 1.0)

            # ---- merged QKV + attention + projection pipeline ----
            # The PE executes its queue IN ORDER, so emitting all of QKV
            # before attention serializes them (~63us of QKV before the
            # first exp).  Instead: a minimal QKV prefix, then the
            # remaining QKV tiles flow through a deadline-ordered work
            # queue sprinkled into the attention i-loops.  PSUM rings
            # (8 banks): s 2x2, y 2x1, aux 2x1 (qkv/proj/warmup/bcast).
            with tc.tile_pool(name="s_ps", bufs=2, space="PSUM") as s_pool, \
                 tc.tile_pool(name="y_ps", bufs=2, space="PSUM") as y_pool, \
                 tc.tile_pool(name="aux_ps", bufs=2, space="PSUM") as aux_pool, \
                 tc.tile_pool(name="dram_sc", bufs=1, space="DRAM") as dram_pool:
                den_dram = dram_pool.tile([NQC * HL, 512], F32, name="den_dram")
                rc_dram = dram_pool.tile([NQC * HL, 512], F32, name="rc_dram")

                # PE warm-up: the HAM clock gate only reaches 8/8 after
                # ~3.4us of sustained activity and the first real matmul
                # can't start until ~4us of input DMA has landed; burn the
                # window on throwaway matmuls over the first-loaded mask.
                wsc_sb = misc.tile([P, 512], BF16, name="wsc_sb", tag="wsc")
                nc.vector.memset(wsc_sb[:, :], 0.0)
                wps = aux_pool.tile([P, 512], F32, name="warmps", tag="aux")
                for w in range(7):
                    nc.tensor.matmul(
                        wps[:, :], mk_sb[:, :], wsc_sb[:, :],
                        start=True, stop=True,
                    )

                def warm_fill(n):
                    # throwaway matmuls that keep the HAM duty-cycle high
                    # across a known PE bubble (dependency-free, so they
                    # execute exactly when the queue would otherwise stall)
                    w_ps = aux_pool.tile([P, 512], F32, name="wfps", tag="aux")
                    for _ in range(n):
                        nc.tensor.matmul(
                            w_ps[:, :], mk_sb[:, :], wsc_sb[:, :],
                            start=True, stop=True,
                        )

                def qk_tile(w_sb, dst_sb, oc, tch):
                    ps = aux_pool.tile([P, 512], F32, name="qkps", tag="aux")
                    for kc in range(KC):
                        nc.tensor.matmul(
                            ps[:, :],
                            w_sb[:, kc * OL + oc * P: kc * OL + oc * P + P],
                            xT_sb[:, kc * T + tch * 512: kc * T + tch * 512 + 512],
                            start=(kc == 0),
                            stop=(kc == KC - 1),
                        )
                    # DVE eviction: ScalarE runs nothing but exp
                    nc.vector.tensor_copy(
                        dst_sb[:, oc * T + tch * 512: oc * T + tch * 512 + 512],
                        ps[:, :],
                    )

                def v_tile(tt):
                    ps = aux_pool.tile([P, 512], F32, name="vps", tag="aux")
                    for kc in range(KC):
                        nc.tensor.matmul(
                            ps[:, 0:OL],
                            xT_sb[:, kc * T + tt * P: kc * T + tt * P + P],
                            wv_sb[:, kc * OL:(kc + 1) * OL],
                            start=(kc == 0),
                            stop=(kc == KC - 1),
                        )
                    nc.vector.tensor_copy(
                        va_view[:, tt, :, 0:D],
                        ps[:, 0:OL].rearrange("p (h d) -> p h d", h=HL),
                    )

                def proj_tile(tile_idx, fine=False):
                    tt, ocn = divmod(tile_idx, 2)
                    trow = tt * P
                    pr_ps = aux_pool.tile([P, 512], F32, name="prps", tag="aux")
                    for cc in range(2):
                        nc.tensor.matmul(
                            pr_ps[:, :],
                            yT_sb[:, cc * T + trow: cc * T + trow + P],
                            wp_sb[:, cc * C + ocn * 512: cc * C + ocn * 512 + 512],
                            start=(cc == 0),
                            stop=(cc == 1),
                        )
                    o_sb = outp.tile([P, 512], F32, name="osb", tag="osb")
                    if not fine:
                        nc.vector.tensor_copy(o_sb[:, :], pr_ps[:, :])
                        # alternate DMA-issuing engine: descriptors land in
                        # two hardware queues, so the 256KB output tiles
                        # drain in parallel instead of serializing
                        if tile_idx % 2 == 0:
                            nc.sync.dma_start(
                                out=out[trow:trow + P, ocn * 512:(ocn + 1) * 512],
                                in_=o_sb[:, :],
                            )
                        else:
                            nc.scalar.dma_start(
                                out=out[trow:trow + P, ocn * 512:(ocn + 1) * 512],
                                in_=o_sb[:, :],
                            )
                    else:
                        # final burst: halve the eviction + DMA granularity
                        # and fan the halves across two queues so the last
                        # bytes leave as early as possible
                        for hf, eng in ((0, nc.sync), (1, nc.scalar)):
                            nc.vector.tensor_copy(
                                o_sb[:, hf * 256:(hf + 1) * 256],
                                pr_ps[:, hf * 256:(hf + 1) * 256],
                            )
                            eng.dma_start(
                                out=out[trow:trow + P,
                                        ocn * 512 + hf * 256: ocn * 512 + (hf + 1) * 256],
                                in_=o_sb[:, hf * 256:(hf + 1) * 256],
                            )

                # minimal prefix: exactly what attention chunk (0,0)'s first
                # S matmul needs (v tiles flow through the work queue -- the
                # first Y matmul only runs ~2 exps later)
                qk_tile(wq_sb, qT_sb, 0, 0)
                qk_tile(wk_sb, kT_sb, 0, 0)

                # the rest of QKV, deadline-ordered by the first chunk that
                # consumes each tile; popped one per attention iteration
                def mk_qk(w_sb, dst_sb, oc, tch):
                    return lambda: qk_tile(w_sb, dst_sb, oc, tch)

                def mk_v(tt):
                    return lambda: v_tile(tt)

                work = [mk_v(0), mk_v(1), mk_v(2), mk_v(3),
                        mk_qk(wq_sb, qT_sb, 1, 0), mk_qk(wk_sb, kT_sb, 1, 0)]
                for tch in (1, 2, 3):
                    work += [mk_qk(wq_sb, qT_sb, 0, tch),
                             mk_qk(wk_sb, kT_sb, 0, tch)]
                    work += [mk_v(tt) for tt in range(4 * tch, 4 * tch + 4)]
                    work += [mk_qk(wq_sb, qT_sb, 1, tch),
                             mk_qk(wk_sb, kT_sb, 1, tch)]
                # units that must be emitted before chunk (j4, hp) starts
                req = {(0, 0): 0, (0, 1): 6, (1, 0): 12, (1, 1): 14,
                       (2, 0): 20, (2, 1): 22, (3, 0): 28, (3, 1): 30}
                seq = [(a, b) for a in range(NQC) for b in range(2)]
                emitted = [0]

                def pop_work():
                    if work:
                        work.pop(0)()
                        emitted[0] += 1

                for j4 in range(NQC):
                    q0 = j4 * 512
                    for hp in range(2):
                        # flush any not-yet-emitted prerequisites
                        while emitted[0] < req[(j4, hp)]:
                            pop_work()
                        nxt = seq.index((j4, hp)) + 1
                        req_next = req[seq[nxt]] if nxt < len(seq) else 30
                        # previous chunk's projection tiles are sprinkled
                        # into the i-loop below: each proj MM is independent
                        # PE work that fills the S->exp->Y handoff bubble.
                        pend = (
                            [(j4 - 1) * 8 + hp * 4 + k for k in range(4)]
                            if j4 > 0 else []
                        )
                        # two heads interleaved per k-tile: one shared 2-bank
                        # S tile, one wide exp for both heads (the +352cyc
                        # ACTIVATE pipeline fill amortizes over 1024 cols),
                        # two independent y accumulations.  Doubles the
                        # PE-side work available per ACT op.
                        h0, h1 = 2 * hp, 2 * hp + 1
                        ch = hp
                        y0 = y_pool.tile([P, 512], F32, name="yps0", tag="yps")
                        y1 = y_pool.tile([P, 512], F32, name="yps1", tag="yps")
                        nk = 4 * (j4 + 1)

                        def emit_y(c0, p2, i):
                            for half, y_ps, hh in ((0, y0, h0), (1, y1, h1)):
                                nc.tensor.matmul(
                                    y_ps[0:VA, c0:512],
                                    va_sb[:, (i * HL + hh) * VA:(i * HL + hh) * VA + VA],
                                    p2[:, half * 512 + c0: half * 512 + 512],
                                    start=(i == 0),
                                    stop=(i == nk - 1),
                                )

                        prev_y = None
                        for i in range(nk):
                            m0 = max(0, i - 4 * j4)
                            c0 = P * m0
                            s2 = s_pool.tile([P, 1024], F32, name="sps", tag="sps")
                            for half, po in ((0, 0), (1, 64)):
                                nc.tensor.matmul(
                                    s2[:, half * 512 + c0: half * 512 + 512],
                                    kT_sb[po:po + D, ch * T + i * P: ch * T + i * P + P],
                                    qT_sb[po:po + D, ch * T + q0 + c0: ch * T + q0 + 512],
                                    start=True,
                                    stop=True,
                                )
                            p2 = att.tile([P, 1024], BF16, name="pt", tag="pt")
                            if m0 == 0:
                                nc.scalar.activation(
                                    p2[:, 0:1024], s2[:, 0:1024], Exp, scale=0.125
                                )
                            else:
                                # diagonal: the two live spans are disjoint;
                                # one 3D-AP exp covers both (halves the
                                # +352cyc ACTIVATE fills on the diagonal)
                                s2v = s2[:, :].rearrange("p (h c) -> p h c", h=2)
                                p2v = p2[:, :].rearrange("p (h c) -> p h c", h=2)
                                nc.scalar.activation(
                                    p2v[:, :, c0:512], s2v[:, :, c0:512],
                                    Exp, scale=0.125,
                                )
                            if i >= 4 * j4:
                                for half in range(2):
                                    nc.gpsimd.tensor_mul(
                                        p2[:, half * 512 + c0: half * 512 + c0 + P],
                                        p2[:, half * 512 + c0: half * 512 + c0 + P],
                                        mk_sb[:, :],
                                    )
                            # independent PE work between S(i) and Y(i-1):
                            # a QKV tile (paced so each chunk's inputs are
                            # ready one chunk ahead) or a proj tile.  These
                            # MMs execute while exp(i-1)/exp(i) run, so the
                            # in-order PE queue never parks on a Y waiting
                            # for its exp.
                            if emitted[0] < req_next:
                                pop_work()
                            elif pend and i % 2 == 1:
                                proj_tile(pend.pop(0))
                            elif work and i % 2 == 0:
                                pop_work()
                            if prev_y is not None:
                                emit_y(*prev_y)
                            prev_y = (c0, p2, i)
                        emit_y(*prev_y)
                        for t in pend:
                            proj_tile(t)

                        # tail: evict both heads' y (frees PSUM), push the two
                        # denominator rows to DRAM, pull them back spread over
                        # 8 partitions, one 8-lane iterative-divide reciprocal
                        # (~0.9us for both heads vs 3.3us/head single-lane),
                        # push back, broadcast-DMA, one DVE multiply per head.
                        # Everything between the eviction copy and the final
                        # multiply runs on DMA queues, off every engine.
                        # final chunk: the projection burst (and kernel end)
                        # gate on this tail's yT rows; warm-filler matmuls
                        # bridge the tail's DMA-chain latency so the burst
                        # runs at full clock.
                        if j4 == NQC - 1 and hp == 1:
                            warm_fill(26)
                        # tail-chain DMAs for the last chunk go through the
                        # gpsimd hardware queue: the sync/scalar queues are
                        # backed up with 256KB output-tile transfers by then,
                        # which was adding ~8us of queueing delay per hop.
                        dq = nc.gpsimd if j4 == NQC - 1 else nc.sync
                        pslot = 2 * (j4 * 2 + hp)
                        y_sbs = []
                        for half, y_ps, hh in ((0, y0, h0), (1, y1, h1)):
                            y_sb = misc.tile([P, 512], F32, name="ysb", tag="ysb")
                            nc.vector.tensor_copy(y_sb[0:VA, :], y_ps[0:VA, :])
                            dq.dma_start(
                                out=den_dram[pslot + half: pslot + half + 1, :],
                                in_=y_sb[D:VA, :],
                            )
                            y_sbs.append(y_sb)
                        dsp = misc.tile([8, 128], F32, name="dsp", tag="dsp")
                        dq.dma_start(
                            out=dsp[:, :],
                            in_=den_dram[pslot: pslot + 2, :].rearrange(
                                "a (p b) -> (a p) b", p=4
                            ),
                        )
                        rc8 = misc.tile([8, 128], F32, name="rc8", tag="rc8")
                        nc.vector.reciprocal(rc8[:, :], dsp[:, :])
                        dq.dma_start(
                            out=rc_dram[pslot: pslot + 2, :].rearrange(
                                "a (p b) -> (a p) b", p=4
                            ),
                            in_=rc8[:, :],
                        )
                        for half, hh in ((0, h0), (1, h1)):
                            po = 64 * half
                            bc_sb = misc.tile([P, 512], F32, name="bcsb", tag="bcsb")
                            dq.dma_start(
                                out=bc_sb[:, :],
                                in_=rc_dram[pslot + half: pslot + half + 1, :]
                                .to_broadcast((P, 512)),
                            )
                            nc.vector.tensor_mul(
                                yT_sb[po:po + D, ch * T + q0: ch * T + q0 + 512],
                                y_sbs[half][0:D, :],
                                bc_sb[0:D, :],
                            )
                        if j4 == NQC - 1 and hp == 1:
                            for k in range(8):
                                proj_tile(j4 * 8 + k, fine=(k >= 4))
    nc.compile()
    return nc


def _get_nc():
    global _nc_cache
    if _nc_cache is None:
        _nc_cache = _build_nc()
    return _nc_cache


def _prepare_in_maps(x, W_qkv, W_proj):
    x = np.asarray(x, np.float32)
    W_qkv = np.asarray(W_qkv, np.float32)
    W_proj = np.asarray(W_proj, np.float32)
    # [r, j] = 1 where j >= r (upper triangle incl diag, in S^T [k, q] layout)
    tri = (np.arange(P)[None, :] >= np.arange(P)[:, None]).astype(np.float32)
    tri = tri.astype(_BF16)
    in_maps = []
    for c in range(8):
        b, g = c // 4, c % 4
        r0 = OL * g
        def img(a):
            # [R, Y] with R = n*128 rows -> SBUF image [128, n*Y]
            n = a.shape[0] // P
            return np.ascontiguousarray(
                a.reshape(n, P, a.shape[1]).transpose(1, 0, 2).reshape(P, -1)
            ).astype(_BF16)

        in_maps.append({
            "xT": img(x[b].T),
            "wqT": img(W_qkv[r0:r0 + OL, :].T),
            "wkT": img(W_qkv[C + r0:C + r0 + OL, :].T),
            "wvT": img(W_qkv[2 * C + r0:2 * C + r0 + OL, :].T),
            "wpT": img(W_proj[:, r0:r0 + OL].T),
            "mask_tri": tri,
        })
    return in_maps


def _combine(results):
    out = np.zeros((B, T, C), np.float32)
    for c in range(8):
        out[c // 4] += results[c]["out"]
    return out


def kernel(x, W_qkv, W_proj):
    nc = _get_nc()
    in_maps = _prepare_in_maps(x, W_qkv, W_proj)
    try:
        res = bass_utils.run_bass_kernel_spmd(nc, in_maps, core_ids=list(range(8)))
    except Exception:
        # rare transient NRT device errors; one retry
        res = bass_utils.run_bass_kernel_spmd(nc, in_maps, core_ids=list(range(8)))
    return _combine(res.results)


def kernel_traced(x, W_qkv, W_proj, trace_cores=None):
    """Like kernel() but returns (out, exec_time_ns) using an NTFF profile."""
    nc = _get_nc()
    in_maps = _prepare_in_maps(x, W_qkv, W_proj)
    res = bass_utils.run_bass_kernel_spmd(
        nc, in_maps, core_ids=list(range(8)), trace=True, trace_cores=trace_cores
    )
    return _combine(res.results), res.exec_time_ns



# revision 55
# speedup vs baseline: 1.0303x; 1.0049x over previous
"""Trainium2 Bass kernel: causal self-attention (B=2, T=2048, C=1024, H=16, Dh=64).

Sharding: 8 cores = 2 (batch) x 4 (head groups of 4 heads).  Each core gets
x[b] plus the W_qkv rows / W_proj columns for its heads, computes the full
attention + a partial output projection for its batch, and the host sums the
4 partials per batch (tensor-parallel unshard).

All matmuls run in bf16 with f32 PSUM accumulation.  x is passed transposed
(xT = x[b].T) so that:
  qT, kT = Wq @ xT, Wk @ xT     (head dim on partitions)  -- no transposes
  v      = xT.T @ WvT           (natural [T, d] layout)
  S^T    = kT_h(tile).T @ qT_h  ([k, q] layout, 128x512 blocks, the two
           heads of a pair row-tiled into array halves -> concurrent)
  exp on ScalarE (logits are bounded, no max pass needed); causal masking by
  computing only the live columns of each block plus one multiplicative
  [128,128] triangle mask on the diagonal subtile (gpsimd); row sums via a
  ones column appended to V (P@[V|1] accumulates y^T and the softmax
  denominators in one PSUM tile).
  out_partial = y^T.T @ WpT   (f32, DMA'd out).

Schedule: the PE executes its queue IN ORDER, so QKV tiles are emitted
through a deadline-ordered work queue sprinkled one-per-iteration into the
attention i-loops (plus previous-chunk projection tiles), never as a bulk
phase.  This keeps the PE continuously busy from ~10us on: the S->exp->Y
handoff bubbles are filled with independent QKV/projection matmuls, which
also keeps the HAM activity monitor at K=8/8 (full 2.4GHz clock) instead of
oscillating into the 1.2GHz throttle state.  Y matmuls are emitted one
iteration behind their exp (software pipelining) so the in-order PE queue
never parks on an unfinished ACTIVATE.

Softmax normalization: denominator rows go through a DRAM round-trip that
re-spreads the 512 q-columns over 8 SBUF partitions, so the iterative-
divide DVE RECIPROCAL runs 8 lanes wide (0.9us for two heads vs 3.3us per
head single-lane); the reciprocal row is broadcast back over 128 partitions
by a stride-0 DRAM-read DMA and applied with one DVE multiply.  The final
chunk's tail gates the last projection burst (and the kernel end), so
throwaway warm-filler matmuls bridge its DMA-chain latency -- the burst
then runs at full clock -- and the burst's last tiles split their output
DMAs across two hardware queues at 256-column granularity.

ScalarE runs nothing but Exp (plus two DMA descriptor issues per projection
pair); every PSUM eviction lives on the DVE.  Diagonal-block exps cover
both disjoint live spans with one 3D-AP ACTIVATE, halving the +352-cycle
pipeline-fill cost on the diagonal.
"""
import sys
import types

import numpy as np
import ml_dtypes

_BF16 = ml_dtypes.bfloat16


def _install_ntff_hook():
    """Provide antenv.axon_hooks so run_bass_kernel_spmd(trace=True) works."""
    if "antenv.axon_hooks" in sys.modules:
        return
    mod = types.ModuleType("antenv.axon_hooks")
    mod._hook = None

    def set_axon_ntff_profile_hook(h):
        mod._hook = h

    def get_axon_ntff_profile_hook():
        return mod._hook

    mod.set_axon_ntff_profile_hook = set_axon_ntff_profile_hook
    mod.get_axon_ntff_profile_hook = get_axon_ntff_profile_hook
    sys.modules["antenv.axon_hooks"] = mod
    try:
        import antenv

        antenv.axon_hooks = mod
    except Exception:
        pass
    try:
        from trn_agent_boot.trn_boot import _ntff_profile_via_ctypes

        mod.set_axon_ntff_profile_hook(
            _ntff_profile_via_ctypes("/opt/axon/libaxon_pjrt.so")
        )
    except Exception:
        pass


_install_ntff_hook()

import concourse.bacc as bacc
import concourse.mybir as mybir
from concourse import bass_utils
from concourse.tile import TileContext

# no network bucket in this container; keep artifacts local
bass_utils.upload_artifacts = lambda tmpdir: tmpdir

BF16 = mybir.dt.bfloat16
F32 = mybir.dt.float32

B, T, C = 2, 2048, 1024
H, D = 16, 64
HL = 4            # heads per core
OL = HL * D       # 256 local qkv output dim
P = 128
KC = C // P       # 8 contraction chunks
NQT = T // P      # 16 q/k 128-tiles
NQC = T // 512    # 4 q 512-chunks
VA = D + 1        # v columns per head incl. ones column (65)

_nc_cache = None


def _build_nc():
    nc = bacc.Bacc("TRN2", target_bir_lowering=False, debug=False, num_devices=8)

    # all inputs arrive pre-arranged in SBUF-image layout [128, X] so every
    # input DMA moves multi-KB contiguous runs per partition row.
    xT = nc.declare_dram_parameter("xT", [P, KC * T], BF16, isOutput=False)
    wqT = nc.declare_dram_parameter("wqT", [P, KC * OL], BF16, isOutput=False)
    wkT = nc.declare_dram_parameter("wkT", [P, KC * OL], BF16, isOutput=False)
    wvT = nc.declare_dram_parameter("wvT", [P, KC * OL], BF16, isOutput=False)
    wpT = nc.declare_dram_parameter("wpT", [P, 2 * C], BF16, isOutput=False)
    mk = nc.declare_dram_parameter("mask_tri", [P, P], BF16, isOutput=False)
    out = nc.declare_dram_parameter("out", [T, C], F32, isOutput=True)

    Exp = mybir.ActivationFunctionType.Exp

    with TileContext(nc) as tc:
        with tc.tile_pool(name="const", bufs=1) as const, \
             tc.tile_pool(name="misc", bufs=3) as misc, \
             tc.tile_pool(name="att", bufs=8) as att, \
             tc.tile_pool(name="outp", bufs=6) as outp:
            xT_sb = const.tile([P, KC * T], BF16, name="xT_sb")
            wq_sb = const.tile([P, KC * OL], BF16, name="wq_sb")
            wk_sb = const.tile([P, KC * OL], BF16, name="wk_sb")
            wv_sb = const.tile([P, KC * OL], BF16, name="wv_sb")
            wp_sb = const.tile([P, 2 * C], BF16, name="wp_sb")
            mk_sb = const.tile([P, P], BF16, name="mk_sb")
            qT_sb = const.tile([P, 2 * T], BF16, name="qT_sb")
            kT_sb = const.tile([P, 2 * T], BF16, name="kT_sb")
            va_sb = const.tile([P, NQT * HL * VA], BF16, name="va_sb")
            yT_sb = const.tile([P, 2 * T], BF16, name="yT_sb")

            # ---- input DMAs: everything is a straight [128, X] image copy.
            # mask first (tiny) -- it feeds the PE warm-up matmuls below.
            # The prefix working set (wq + first x pieces + wk) is split
            # across FOUR issuing engines so it lands in parallel hardware
            # queues instead of serializing ~6MB behind one queue.
            nc.sync.dma_start(out=mk_sb[:, :], in_=mk[:, :])
            nc.sync.dma_start(out=wq_sb[:, :], in_=wqT[:, :])
            # each sync-queue DMA issue costs ~0.6us; batching the 8 x
            # pieces per half into 2 strided 3D-AP transfers cuts the
            # issue serialization that was pacing the first QKV matmuls
            xv = xT_sb[:, :].rearrange("p (n t) -> p n t", n=KC)
            xs = xT[:, :].rearrange("p (n t) -> p n t", n=KC)
            nc.sync.dma_start(out=xv[:, 0:4, 0:512], in_=xs[:, 0:4, 0:512])
            nc.sync.dma_start(out=xv[:, 4:8, 0:512], in_=xs[:, 4:8, 0:512])
            nc.sync.dma_start(out=wk_sb[:, :], in_=wkT[:, :])
            nc.sync.dma_start(out=wv_sb[:, :], in_=wvT[:, :])
            nc.sync.dma_start(out=xv[:, 0:4, 512:T], in_=xs[:, 0:4, 512:T])
            nc.sync.dma_start(out=xv[:, 4:8, 512:T], in_=xs[:, 4:8, 512:T])
            nc.sync.dma_start(out=wp_sb[:, :], in_=wpT[:, :])
            va_view = va_sb[:, :].rearrange("p (t h e) -> p t h e", t=NQT, h=HL)
            nc.vector.memset(va_view[:, :, :, D:VA], 1.0)

            # ---- merged QKV + attention + projection pipeline ----
            # The PE executes its queue IN ORDER, so emitting all of QKV
            # before attention serializes them (~63us of QKV before the
            # first exp).  Instead: a minimal QKV prefix, then the
            # remaining QKV tiles flow through a deadline-ordered work
            # queue sprinkled into the attention i-loops.  PSUM rings
            # (8 banks): s 2x2, y 2x1, aux 2x1 (qkv/proj/warmup/bcast).
            with tc.tile_pool(name="s_ps", bufs=2, space="PSUM") as s_pool, \
                 tc.tile_pool(name="y_ps", bufs=2, space="PSUM") as y_pool, \
                 tc.tile_pool(name="aux_ps", bufs=2, space="PSUM") as aux_pool, \
                 tc.tile_pool(name="dram_sc", bufs=1, space="DRAM") as dram_pool:
                den_dram = dram_pool.tile([NQC * HL, 512], F32, name="den_dram")
                rc_dram = dram_pool.tile([NQC * HL, 512], F32, name="rc_dram")

                # PE warm-up: the HAM clock gate only reaches 8/8 after
                # ~3.4us of sustained activity and the first real matmul
                # can't start until ~4us of input DMA has landed; burn the
                # window on throwaway matmuls over the first-loaded mask.
                wsc_sb = misc.tile([P, 512], BF16, name="wsc_sb", tag="wsc")
                nc.vector.memset(wsc_sb[:, :], 0.0)
                wps = aux_pool.tile([P, 512], F32, name="warmps", tag="aux")
                for w in range(7):
                    nc.tensor.matmul(
                        wps[:, :], mk_sb[:, :], wsc_sb[:, :],
                        start=True, stop=True,
                    )

                def warm_fill(n):
                    # throwaway matmuls that keep the HAM duty-cycle high
                    # across a known PE bubble (dependency-free, so they
                    # execute exactly when the queue would otherwise stall)
                    w_ps = aux_pool.tile([P, 512], F32, name="wfps", tag="aux")
                    for _ in range(n):
                        nc.tensor.matmul(
                            w_ps[:, :], mk_sb[:, :], wsc_sb[:, :],
                            start=True, stop=True,
                        )

                def qk_tile(w_sb, dst_sb, oc, tch):
                    ps = aux_pool.tile([P, 512], F32, name="qkps", tag="aux")
                    for kc in range(KC):
                        nc.tensor.matmul(
                            ps[:, :],
                            w_sb[:, kc * OL + oc * P: kc * OL + oc * P + P],
                            xT_sb[:, kc * T + tch * 512: kc * T + tch * 512 + 512],
                            start=(kc == 0),
                            stop=(kc == KC - 1),
                        )
                    # DVE eviction: ScalarE runs nothing but exp
                    nc.vector.tensor_copy(
                        dst_sb[:, oc * T + tch * 512: oc * T + tch * 512 + 512],
                        ps[:, :],
                    )

                def v_tile(tt):
                    ps = aux_pool.tile([P, 512], F32, name="vps", tag="aux")
                    for kc in range(KC):
                        nc.tensor.matmul(
                            ps[:, 0:OL],
                            xT_sb[:, kc * T + tt * P: kc * T + tt * P + P],
                            wv_sb[:, kc * OL:(kc + 1) * OL],
                            start=(kc == 0),
                            stop=(kc == KC - 1),
                        )
                    nc.vector.tensor_copy(
                        va_view[:, tt, :, 0:D],
                        ps[:, 0:OL].rearrange("p (h d) -> p h d", h=HL),
                    )

                def proj_tile(tile_idx, fine=False):
                    tt, ocn = divmod(tile_idx, 2)
                    trow = tt * P
                    pr_ps = aux_pool.tile([P, 512], F32, name="prps", tag="aux")
                    for cc in range(2):
                        nc.tensor.matmul(
                            pr_ps[:, :],
                            yT_sb[:, cc * T + trow: cc * T + trow + P],
                            wp_sb[:, cc * C + ocn * 512: cc * C + ocn * 512 + 512],
                            start=(cc == 0),
                            stop=(cc == 1),
                        )
                    o_sb = outp.tile([P, 512], F32, name="osb", tag="osb")
                    if not fine:
                        nc.vector.tensor_copy(o_sb[:, :], pr_ps[:, :])
                        # alternate DMA-issuing engine: descriptors land in
                        # two hardware queues, so the 256KB output tiles
                        # drain in parallel instead of serializing
                        if tile_idx % 2 == 0:
                            nc.sync.dma_start(
                                out=out[trow:trow + P, ocn * 512:(ocn + 1) * 512],
                                in_=o_sb[:, :],
                            )
                        else:
                            nc.scalar.dma_start(
                                out=out[trow:trow + P, ocn * 512:(ocn + 1) * 512],
                                in_=o_sb[:, :],
                            )
                    else:
                        # final burst: halve the eviction + DMA granularity
                        # and fan the halves across two queues so the last
                        # bytes leave as early as possible
                        for hf, eng in ((0, nc.sync), (1, nc.scalar)):
                            nc.vector.tensor_copy(
                                o_sb[:, hf * 256:(hf + 1) * 256],
                                pr_ps[:, hf * 256:(hf + 1) * 256],
                            )
                            eng.dma_start(
                                out=out[trow:trow + P,
                                        ocn * 512 + hf * 256: ocn * 512 + (hf + 1) * 256],
                                in_=o_sb[:, hf * 256:(hf + 1) * 256],
                            )

                # minimal prefix: exactly what attention chunk (0,0)'s first
                # S matmul needs (v tiles flow through the work queue -- the
                # first Y matmul only runs ~2 exps later)
                qk_tile(wq_sb, qT_sb, 0, 0)
                qk_tile(wk_sb, kT_sb, 0, 0)

                # the rest of QKV, deadline-ordered by the first chunk that
                # consumes each tile; popped one per attention iteration
                def mk_qk(w_sb, dst_sb, oc, tch):
                    return lambda: qk_tile(w_sb, dst_sb, oc, tch)

                def mk_v(tt):
                    return lambda: v_tile(tt)

                work = [mk_v(0), mk_v(1), mk_v(2), mk_v(3),
                        mk_qk(wq_sb, qT_sb, 1, 0), mk_qk(wk_sb, kT_sb, 1, 0)]
                for tch in (1, 2, 3):
                    work += [mk_qk(wq_sb, qT_sb, 0, tch),
                             mk_qk(wk_sb, kT_sb, 0, tch)]
                    work += [mk_v(tt) for tt in range(4 * tch, 4 * tch + 4)]
                    work += [mk_qk(wq_sb, qT_sb, 1, tch),
                             mk_qk(wk_sb, kT_sb, 1, tch)]
                # units that must be emitted before chunk (j4, hp) starts
                req = {(0, 0): 0, (0, 1): 6, (1, 0): 12, (1, 1): 14,
                       (2, 0): 20, (2, 1): 22, (3, 0): 28, (3, 1): 30}
                seq = [(a, b) for a in range(NQC) for b in range(2)]
                emitted = [0]

                def pop_work():
                    if work:
                        work.pop(0)()
                        emitted[0] += 1

                for j4 in range(NQC):
                    q0 = j4 * 512
                    for hp in range(2):
                        # flush any not-yet-emitted prerequisites
                        while emitted[0] < req[(j4, hp)]:
                            pop_work()
                        nxt = seq.index((j4, hp)) + 1
                        req_next = req[seq[nxt]] if nxt < len(seq) else 30
                        # previous chunk's projection tiles are sprinkled
                        # into the i-loop below: each proj MM is independent
                        # PE work that fills the S->exp->Y handoff bubble.
                        pend = (
                            [(j4 - 1) * 8 + hp * 4 + k for k in range(4)]
                            if j4 > 0 else []
                        )
                        # two heads interleaved per k-tile: one shared 2-bank
                        # S tile, one wide exp for both heads (the +352cyc
                        # ACTIVATE pipeline fill amortizes over 1024 cols),
                        # two independent y accumulations.  Doubles the
                        # PE-side work available per ACT op.
                        h0, h1 = 2 * hp, 2 * hp + 1
                        ch = hp
                        y0 = y_pool.tile([P, 512], F32, name="yps0", tag="yps")
                        y1 = y_pool.tile([P, 512], F32, name="yps1", tag="yps")
                        nk = 4 * (j4 + 1)

                        def emit_y(c0, p2, i):
                            for half, y_ps, hh in ((0, y0, h0), (1, y1, h1)):
                                nc.tensor.matmul(
                                    y_ps[0:VA, c0:512],
                                    va_sb[:, (i * HL + hh) * VA:(i * HL + hh) * VA + VA],
                                    p2[:, half * 512 + c0: half * 512 + 512],
                                    start=(i == 0),
                                    stop=(i == nk - 1),
                                )

                        prev_y = None
                        for i in range(nk):
                            m0 = max(0, i - 4 * j4)
                            c0 = P * m0
                            s2 = s_pool.tile([P, 1024], F32, name="sps", tag="sps")
                            for half, po in ((0, 0), (1, 64)):
                                nc.tensor.matmul(
                                    s2[:, half * 512 + c0: half * 512 + 512],
                                    kT_sb[po:po + D, ch * T + i * P: ch * T + i * P + P],
                                    qT_sb[po:po + D, ch * T + q0 + c0: ch * T + q0 + 512],
                                    start=True,
                                    stop=True,
                                )
                            p2 = att.tile([P, 1024], BF16, name="pt", tag="pt")
                            if m0 == 0:
                                nc.scalar.activation(
                                    p2[:, 0:1024], s2[:, 0:1024], Exp, scale=0.125
                                )
                            else:
                                # diagonal: the two live spans are disjoint;
                                # one 3D-AP exp covers both (halves the
                                # +352cyc ACTIVATE fills on the diagonal)
                                s2v = s2[:, :].rearrange("p (h c) -> p h c", h=2)
                                p2v = p2[:, :].rearrange("p (h c) -> p h c", h=2)
                                nc.scalar.activation(
                                    p2v[:, :, c0:512], s2v[:, :, c0:512],
                                    Exp, scale=0.125,
                                )
                            if i >= 4 * j4:
                                for half in range(2):
                                    nc.gpsimd.tensor_mul(
                                        p2[:, half * 512 + c0: half * 512 + c0 + P],
                                        p2[:, half * 512 + c0: half * 512 + c0 + P],
                                        mk_sb[:, :],
                                    )
                            # independent PE work between S(i) and Y(i-1):
                            # a QKV tile (paced so each chunk's inputs are
                            # ready one chunk ahead) or a proj tile.  These
                            # MMs execute while exp(i-1)/exp(i) run, so the
                            # in-order PE queue never parks on a Y waiting
                            # for its exp.
                            if emitted[0] < req_next:
                                pop_work()
                            elif pend and i % 2 == 1:
                                proj_tile(pend.pop(0))
                            elif work and i % 2 == 0:
                                pop_work()
                            if prev_y is not None:
                                emit_y(*prev_y)
                            prev_y = (c0, p2, i)
                        emit_y(*prev_y)
                        for t in pend:
                            proj_tile(t)

                        # tail: evict both heads' y (frees PSUM), push the two
                        # denominator rows to DRAM, pull them back spread over
                        # 8 partitions, one 8-lane iterative-divide reciprocal
                        # (~0.9us for both heads vs 3.3us/head single-lane),
                        # push back, broadcast-DMA, one DVE multiply per head.
                        # Everything between the eviction copy and the final
                        # multiply runs on DMA queues, off every engine.
                        # final chunk: the projection burst (and kernel end)
                        # gate on this tail's yT rows; warm-filler matmuls
                        # bridge the tail's DMA-chain latency so the burst
                        # runs at full clock.
                        if j4 == NQC - 1 and hp == 1:
                            warm_fill(26)
                        # tail-chain DMAs for the last chunk go through the
                        # gpsimd hardware queue: the sync/scalar queues are
                        # backed up with 256KB output-tile transfers by then,
                        # which was adding ~8us of queueing delay per hop.
                        dq = nc.gpsimd if j4 == NQC - 1 else nc.sync
                        pslot = 2 * (j4 * 2 + hp)
                        y_sbs = []
                        for half, y_ps, hh in ((0, y0, h0), (1, y1, h1)):
                            y_sb = misc.tile([P, 512], F32, name="ysb", tag="ysb")
                            nc.vector.tensor_copy(y_sb[0:VA, :], y_ps[0:VA, :])
                            dq.dma_start(
                                out=den_dram[pslot + half: pslot + half + 1, :],
                                in_=y_sb[D:VA, :],
                            )
                            y_sbs.append(y_sb)
                        dsp = misc.tile([8, 128], F32, name="dsp", tag="dsp")
                        dq.dma_start(
                            out=dsp[:, :],
                            in_=den_dram[pslot: pslot + 2, :].rearrange(
                                "a (p b) -> (a p) b", p=4
                            ),
                        )
                        rc8 = misc.tile([8, 128], F32, name="rc8", tag="rc8")
                        nc.vector.reciprocal(rc8[:, :], dsp[:, :])
                        dq.dma_start(
                            out=rc_dram[pslot: pslot + 2, :].rearrange(
                                "a (p b) -> (a p) b", p=4
                            ),
                            in_=rc8[:, :],
                        )
                        for half, hh in ((0, h0), (1, h1)):
                            po = 64 * half
                            bc_sb = misc.tile([P, 512], F32, name="bcsb", tag="bcsb")
                            dq.dma_start(
                                out=bc_sb[:, :],
                                in_=rc_dram[pslot + half: pslot + half + 1, :]
                                .to_broadcast((P, 512)),
                            )
                            nc.vector.tensor_mul(
                                yT_sb[po:po + D, ch * T + q0: ch * T + q0 + 512],
                                y_sbs[half][0:D, :],
                                bc_sb[0:D, :],
                            )
                        if j4 == NQC - 1 and hp == 1:
                            for k in range(8):
                                proj_tile(j4 * 8 + k, fine=(k >= 4))
    nc.compile()
    return nc


def _get_nc():
    global _nc_cache
    if _nc_cache is None:
        _nc_cache = _build_nc()
    return _nc_cache


def _prepare_in_maps(x, W_qkv, W_proj):
    x = np.asarray(x, np.float32)
    W_qkv = np.asarray(W_qkv, np.float32)
    W_proj = np.asarray(W_proj, np.float32)
    # [r, j] = 1 where j >= r (upper triangle incl diag, in S^T [k, q] layout)
    tri = (np.arange(P)[None, :] >= np.arange(P)[:, None]).astype(np.float32)
    tri = tri.astype(_BF16)
    in_maps = []
    for c in range(8):
        b, g = c // 4, c % 4
        r0 = OL * g
        def img(a):
            # [R, Y] with R = n*128 rows -> SBUF image [128, n*Y]
            n = a.shape[0] // P
            return np.ascontiguousarray(
                a.reshape(n, P, a.shape[1]).transpose(1, 0, 2).reshape(P, -1)
            ).astype(_BF16)

        in_maps.append({
            "xT": img(x[b].T),
            "wqT": img(W_qkv[r0:r0 + OL, :].T),
            "wkT": img(W_qkv[C + r0:C + r0 + OL, :].T),
            "wvT": img(W_qkv[2 * C + r0:2 * C + r0 + OL, :].T),
            "wpT": img(W_proj[:, r0:r0 + OL].T),
            "mask_tri": tri,
        })
    return in_maps


def _combine(results):
    out = np.zeros((B, T, C), np.float32)
    for c in range(8):
        out[c // 4] += results[c]["out"]
    return out


def kernel(x, W_qkv, W_proj):
    nc = _get_nc()
    in_maps = _prepare_in_maps(x, W_qkv, W_proj)
    try:
        res = bass_utils.run_bass_kernel_spmd(nc, in_maps, core_ids=list(range(8)))
    except Exception:
        # rare transient NRT device errors; one retry
        res = bass_utils.run_bass_kernel_spmd(nc, in_maps, core_ids=list(range(8)))
    return _combine(res.results)


def kernel_traced(x, W_qkv, W_proj, trace_cores=None):
    """Like kernel() but returns (out, exec_time_ns) using an NTFF profile."""
    nc = _get_nc()
    in_maps = _prepare_in_maps(x, W_qkv, W_proj)
    res = bass_utils.run_bass_kernel_spmd(
        nc, in_maps, core_ids=list(range(8)), trace=True, trace_cores=trace_cores
    )
    return _combine(res.results), res.exec_time_ns

